# revision 7
# baseline (speedup 1.0000x reference)
"""Trainium2 Bass kernel for nn_Critic_Mix (dense MLP critic with teacher mixing).

Strategy (v2 — supertile + selective fp8 DoubleRow):
  - Pure data parallel: batch (B=262144) sharded across 8 cores (32768 rows each).
  - 1024-row "supertiles": every PSUM product is a 2-bank [128,1024] tile filled
    by two 512-col matmuls sharing weights; the single fused evacuation per
    product keeps its per-partition bias (both halves are the same product).
    21 evac instructions per supertile, alternated ACT/DVE (11/10).
  - fp8e4 DoubleRow (measured same 216ns issue gap as fp16, 2x contraction):
      * L2 main-path accumulation: teacher pairs (rh1_k, rh1_k+1) -> 2 DR
        matmuls instead of 4 (per head per half).
      * L3 teachers: block-diagonal weights pair (rh2_h0_k, rh2_h1_k) across
        the two heads -> 4 full-width DR matmuls replace 8.
  - Teacher activations rh1/rh2 stored fp8e4 (scaled s1/s2); main path
    (xu, h1, h2, h3, all L1, L2t weights, L3 mains, L4) stays fp16.
    L2t matmuls run mixed fp16-weights x fp8-moving (measured full speed).
  - All scales are powers of two folded into host-packed weights/biases, so
    every evacuation is exactly relu(psum + bias): ACT activation(Relu) and
    DVE tensor_scalar(add,max) are interchangeable.
  - L3 mains: two 64-out fp16 matmuls col-paired via tile_position (measured
    fully concurrent). L4: 16-col block-diag variants accumulate 8 half-tiles
    into one PSUM bank; one small ACT copy (folding 1/S3) + DMA per 8.
  - Full-batch numpy-emulated rel err of this exact scheme: 5.6e-3.
"""

import os
import sys
from contextlib import ExitStack

import ml_dtypes
import numpy as np

for _p in ("/opt/trn_rl_repo",):
    if _p not in sys.path and os.path.isdir(_p):
        sys.path.insert(0, _p)

import concourse.bass as bass
import concourse.tile as tile
from concourse import bacc, mybir
from concourse._compat import with_exitstack
from concourse.bass import ts
from concourse.bass_utils import run_bass_kernel_spmd

# Problem constants (hardcoded; kernel.py must be self-contained).
B = 262144
NCORES = 8
CB = B // NCORES          # rows per core
ST = 1024                 # supertile rows (2 PSUM banks per product)
NT = 512                  # matmul moving width (one PSUM bank)
H1 = 128
H2 = 64
K = 4

F32 = mybir.dt.float32
F16 = mybir.dt.float16
F8 = mybir.dt.float8e4
E4NP = ml_dtypes.float8_e4m3
AF = mybir.ActivationFunctionType
ALU = mybir.AluOpType
PM = mybir.MatmulPerfMode

# power-of-two scales (folded host-side; see docstring)
S1 = 16.0   # rh1 storage scale
S2 = 16.0   # rh2 storage scale
SM = 128.0  # h2 storage scale (L2 main accum)
S3 = 128.0  # h3 storage scale


# ---------------------------------------------------------------------------
# fp16 weight column layout: [128, N16COLS]
# ---------------------------------------------------------------------------
def _w16layout():
    off = {}
    cur = 0
    for h in (0, 1):
        for k in range(K):
            off[f"l1t{h}{k}"] = (cur, 128)
            cur += 128
        off[f"l1m{h}"] = (cur, 128)
        cur += 128
        for k in range(K):
            off[f"l2t{h}{k}"] = (cur, 128)
            cur += 128
        off[f"l2m{h}"] = (cur, 128)
        cur += 128
        off[f"l3m{h}"] = (cur, 64)
        cur += 64
    off["l4"] = (cur, 8 * 16)
    cur += 8 * 16
    return off, cur


W16OFF, N16COLS = _w16layout()

# fp8 weight slots: [128, 8, 2, 128]; slots 0-3 = wacc(h0p0,h0p1,h1p0,h1p1),
# slots 4-7 = w3t k=0..3 (block-diag over heads)
N8SLOTS = 8


def _blayout():
    off = {}
    cur = 0
    for h in (0, 1):
        for k in range(K):
            off[f"b1t{h}{k}"] = cur
            cur += 1
        off[f"b1m{h}"] = cur
        cur += 1
        for k in range(K):
            off[f"b2t{h}{k}"] = cur
            cur += 1
        off[f"b2m{h}"] = cur
        cur += 1
    off["b3cat"] = cur
    cur += 1
    return off, cur


BOFF, BCOLS = _blayout()


# ---------------------------------------------------------------------------
# Host-side parameter folding
# ---------------------------------------------------------------------------
def _q8(x):
    return np.asarray(x, np.float32).astype(E4NP)


def prepare_params(inputs):
    """Returns (w16 [128,N16COLS] f16, w8 [128,8,2,128] f8e4,
    biasv [128,BCOLS] f32, (b4, b8))."""
    m = np.float32(np.asarray(inputs["mix_factor"]).reshape(-1)[0])
    P = np.asarray(inputs["teacher_P"], np.float32).reshape(K)
    om = np.float32(1.0) - m
    c = m * P  # [K], >= 0

    w16 = np.zeros((128, N16COLS), np.float32)
    w8 = np.zeros((128, N8SLOTS, 2, 128), np.float32)
    biasv = np.zeros((128, BCOLS), np.float32)

    def wput(name, arr):
        o, wd = W16OFF[name]
        arr = np.asarray(arr, np.float32)
        assert arr.shape[1] == wd, (name, arr.shape, wd)
        w16[: arr.shape[0], o : o + wd] = arr

    def bput(name, vec, parts=slice(0, 128)):
        vec = np.asarray(vec, np.float32).reshape(-1)
        biasv[parts, BOFF[name]] = vec

    heads = [
        ("W1", "b1", "W2", "b2", "W3", "b3", "W4", "b4", "tW1", "tb1", "tW2", "tb2", "tW3", "tb3"),
        ("W5", "b5", "W6", "b6", "W7", "b7", "W8", "b8", "tW5", "tb5", "tW6", "tb6", "tW7", "tb7"),
    ]
    out_biases = []
    l4 = np.zeros((128, 8 * 16), np.float32)
    for h, names in enumerate(heads):
        (Wa, ba, Wb, bb, Wc, bc, Wd, bd, tWa, tba, tWb, tbb, tWc, tbc) = (
            np.asarray(inputs[n], np.float32) for n in names
        )
        # L1 teachers: weights pre-scaled s1 (psum = s1*z1nb); bias s1*tba.
        for k in range(K):
            wput(f"l1t{h}{k}", (S1 * tWa[k]).T)
            bput(f"b1t{h}{k}", S1 * tba[k])
        # L1 main folded (mixsum of layer 1 is linear in xu).
        W1eff = om * Wa + m * np.einsum("k,koi->oi", P, tWa)
        b1eff = om * ba + m * (P[:, None] * tba).sum(0)
        wput(f"l1m{h}", W1eff.T)
        bput(f"b1m{h}", b1eff)
        # L2 teachers: f16 weights (s2/s1 * c_k tWb) x fp8 rh1; bias s2*c_k*tbb.
        for k in range(K):
            wput(f"l2t{h}{k}", ((S2 / S1) * c[k] * tWb[k]).T)
            bput(f"b2t{h}{k}", S2 * c[k] * tbb[k])
        # L2 main: f16 (SM om Wb) @ h1 + DR fp8 pairs q8(SM c_k tWb / s1) @ rh1.
        wput(f"l2m{h}", (SM * om * Wb).T)
        bput(f"b2m{h}", SM * (om * bb + (c[:, None] * tbb).sum(0)))
        for p in range(2):
            for s in range(2):
                k = 2 * p + s
                w8[:, 2 * h + p, s, :] = (SM * c[k] * tWb[k] / S1).T
        # L3 main: f16 (S3 om Wc / SM) @ (SM h2); stored h3 = S3*h3.
        wput(f"l3m{h}", (S3 * om * Wc / SM).T)
        b3eff = om * bc + m * (P[:, None] * tbc).sum(0)
        bput("b3cat", S3 * b3eff, parts=slice(64 * h, 64 * h + 64))
        # L3 teachers block-diag: slot 4+k slice h covers head h's 64 outputs.
        for k in range(K):
            w8[:, 4 + k, h, 64 * h : 64 * h + 64] = (S3 * tWc[k] / S2).T
        # L4 variants (j = half-tile % 8): psum = S3 * y_partial.
        for j in range(8):
            l4[64 * h : 64 * h + 64, j * 16 + 8 * h + j] = Wd[0]
        out_biases.append(np.float32(bd[0]))

    o, wd = W16OFF["l4"]
    w16[:, o : o + wd] = l4
    return w16.astype(np.float16), _q8(w8), biasv, out_biases


def prepare_xut(inputs):
    x = np.asarray(inputs["x"], np.float32)
    u = np.asarray(inputs["u"], np.float32)
    xu = np.concatenate([x, u], axis=1)  # [B, 128]
    return np.ascontiguousarray(xu.T).astype(np.float16)  # [128, B]


# ---------------------------------------------------------------------------
# Kernel body
# ---------------------------------------------------------------------------
@with_exitstack
def _critic_body(ctx: ExitStack, tc, out_ap, xu_ap, w16_ap, w8_ap, bias_ap, sts: int):
    nc = tc.nc

    const = ctx.enter_context(tc.tile_pool(name="const", bufs=1))
    xup = ctx.enter_context(tc.tile_pool(name="xup", bufs=4))
    actp = ctx.enter_context(tc.tile_pool(name="actp", bufs=3))
    psp = ctx.enter_context(tc.tile_pool(name="psp", bufs=4, space=bass.MemorySpace.PSUM))

    w16 = const.tile([128, N16COLS], F16)
    nc.gpsimd.dma_start(w16[:], w16_ap[:])
    w8 = const.tile([128, N8SLOTS, 2, 128], F8)
    nc.gpsimd.dma_start(w8[:], w8_ap[:])
    biasv = const.tile([128, BCOLS], F32)
    nc.gpsimd.dma_start(biasv[:], bias_ap[:])

    def w(name):
        o, wd = W16OFF[name]
        return w16[:, o : o + wd]

    def bvec(name):
        col = BOFF[name]
        return biasv[:, col : col + 1]

    ecnt = [0]

    def evac(dst, src, bname):
        # dst = relu(src + bias); alternate engines for balance
        if ecnt[0] % 2 == 0:
            nc.scalar.activation(dst, src, AF.Relu, bias=bvec(bname), scale=1.0)
        else:
            nc.vector.tensor_scalar(
                out=dst, in0=src, scalar1=bvec(bname), scalar2=0.0,
                op0=ALU.add, op1=ALU.max,
            )
        ecnt[0] += 1

    l4o, _ = W16OFF["l4"]
    HALF = (slice(0, NT), slice(NT, ST))
    for t in range(sts):
        ecnt[0] = 0
        xu = xup.tile([128, ST], F16, tag="xu")
        nc.gpsimd.dma_start(xu[:], xu_ap[:, ts(t, ST)])

        rh1p = actp.tile([128, 8, ST], F8, tag="rh1p")
        rh2p = actp.tile([128, 8, ST], F8, tag="rh2p")
        h1t = actp.tile([128, 2, ST], F16, tag="h1t")
        h2t = actp.tile([128, 2, ST], F16, tag="h2t")
        h3t = actp.tile([128, ST], F16, tag="h3t")

        # ---- L1: 8 teacher + 2 main products
        for h in (0, 1):
            for k in range(K):
                ps = psp.tile([128, ST], F32, tag="ps")
                for hf in (0, 1):
                    nc.tensor.matmul(ps[:, HALF[hf]], w(f"l1t{h}{k}"), xu[:, HALF[hf]],
                                     start=True, stop=True)
                evac(rh1p[:, 4 * h + k, :], ps[:], f"b1t{h}{k}")
            ps = psp.tile([128, ST], F32, tag="ps")
            for hf in (0, 1):
                nc.tensor.matmul(ps[:, HALF[hf]], w(f"l1m{h}"), xu[:, HALF[hf]],
                                 start=True, stop=True)
            evac(h1t[:, h, :], ps[:], f"b1m{h}")

        # ---- L2 teachers: f16 weights x fp8 rh1 (mixed)
        for h in (0, 1):
            for k in range(K):
                ps = psp.tile([128, ST], F32, tag="ps")
                for hf in (0, 1):
                    nc.tensor.matmul(ps[:, HALF[hf]], w(f"l2t{h}{k}"),
                                     rh1p[:, 4 * h + k, HALF[hf]], start=True, stop=True)
                evac(rh2p[:, 4 * h + k, :], ps[:], f"b2t{h}{k}")

        # ---- L2 main accum: f16 main + 2 DR teacher pairs per head
        for h in (0, 1):
            ps = psp.tile([128, ST], F32, tag="ps")
            for hf in (0, 1):
                nc.tensor.matmul(ps[:, HALF[hf]], w(f"l2m{h}"), h1t[:, h, HALF[hf]],
                                 start=True, stop=False)
                for p in range(2):
                    nc.tensor.matmul(ps[:, HALF[hf]], w8[:, 2 * h + p],
                                     rh1p[:, 4 * h + 2 * p : 4 * h + 2 * p + 2, HALF[hf]],
                                     start=False, stop=(p == 1), perf_mode=PM.DoubleRow)
            evac(h2t[:, h, :], ps[:], f"b2m{h}")

        # ---- L3: 4 block-diag DR teacher slots + col-paired f16 mains
        ps3 = psp.tile([128, ST], F32, tag="ps")
        for hf in (0, 1):
            # full-width DR teachers carry the (tracked) start/stop flags; the
            # col-paired mains are skip_group_check and accumulate in between.
            for k in range(K):
                nc.tensor.matmul(ps3[:, HALF[hf]], w8[:, 4 + k],
                                 rh2p[:, k :: 4, HALF[hf]],
                                 start=(k == 0), stop=(k == K - 1), perf_mode=PM.DoubleRow)
            for h in (0, 1):
                nc.tensor.matmul(ps3[64 * h : 64 * h + 64, HALF[hf]], w(f"l3m{h}"),
                                 h2t[:, h, HALF[hf]], start=False, stop=False,
                                 tile_position=(0, 64 * h), skip_group_check=True)
        evac(h3t[:], ps3[:], "b3cat")

        # ---- L4: per-supertile product in the rotating pool; result rows are
        # 0 (head0) and 8 (head1) of variant j=0; DMA straight from PSUM.
        ps4 = psp.tile([128, ST], F32, tag="ps")
        for hf in (0, 1):
            nc.tensor.matmul(ps4[0:16, HALF[hf]], w16[:, l4o : l4o + 16],
                             h3t[:, HALF[hf]], start=True, stop=True)
        o = actp.tile([16, ST], F32, tag="osb")
        nc.scalar.activation(o[:], ps4[0:16, :], AF.Copy, bias=0.0, scale=1.0 / S3)
        nc.gpsimd.dma_start(out_ap[:, ts(t, ST)], o[:])


def build_nc(cb=CB):
    """Build + compile the per-core program for cb rows (cb % (8*NT) == 0)."""
    assert cb % (8 * NT) == 0
    sts = cb // ST
    nc = bacc.Bacc(
        "TRN2",
        target_bir_lowering=False,
        debug=False,
        enable_asserts=False,
        num_devices=NCORES,
    )
    xu_ap = nc.dram_tensor("xut", [128, cb], F16, kind="ExternalInput").ap()
    w16_ap = nc.dram_tensor("w16", [128, N16COLS], F16, kind="ExternalInput").ap()
    w8_ap = nc.dram_tensor("w8", [128, N8SLOTS, 2, 128], F8, kind="ExternalInput").ap()
    bias_ap = nc.dram_tensor("biasv", [128, BCOLS], F32, kind="ExternalInput").ap()
    out_ap = nc.dram_tensor("out", [16, cb], F32, kind="ExternalOutput").ap()
    with tile.TileContext(nc) as tc:
        _critic_body(tc, out_ap, xu_ap, w16_ap, w8_ap, bias_ap, sts)
    nc.compile()
    return nc


def unscramble_out(out_c):
    """[16, cb] device layout -> (y1 [cb], y2 [cb]); row 8h = head h (S3-scaled)."""
    return [np.asarray(out_c[8 * h], np.float32) for h in (0, 1)]


_NC_CACHE = {}
LAST_RESULTS = None  # BassKernelResults of the most recent run (for profiling)


def kernel(**inputs):
    global LAST_RESULTS
    w16, w8, biasv, (b4, b8) = prepare_params(inputs)
    xut = prepare_xut(inputs)

    if CB not in _NC_CACHE:
        _NC_CACHE[CB] = build_nc(CB)
    nc = _NC_CACHE[CB]

    in_maps = [
        {
            "xut": np.ascontiguousarray(xut[:, c * CB : (c + 1) * CB]),
            "w16": w16,
            "w8": w8,
            "biasv": biasv,
        }
        for c in range(NCORES)
    ]
    res = run_bass_kernel_spmd(
        nc,
        in_maps,
        list(range(NCORES)),
        trace=bool(os.environ.get("BASS_TRACE")),
    )
    LAST_RESULTS = res

    y1 = np.empty(B, np.float32)
    y2 = np.empty(B, np.float32)
    for c in range(NCORES):
        a, b = unscramble_out(res.results[c]["out"])
        y1[c * CB : (c + 1) * CB] = a
        y2[c * CB : (c + 1) * CB] = b
    y1 += b4
    y2 += b8
    return (y1[:, None], y2[:, None])


# revision 9
# speedup vs baseline: 1.1375x; 1.1375x over previous
"""Trainium2 Bass kernel for nn_Critic_Mix (dense MLP critic with teacher mixing).

Strategy (v2 — supertile + selective fp8 DoubleRow):
  - Pure data parallel: batch (B=262144) sharded across 8 cores (32768 rows each).
  - 1024-row "supertiles": every PSUM product is a 2-bank [128,1024] tile filled
    by two 512-col matmuls sharing weights; the single fused evacuation per
    product keeps its per-partition bias (both halves are the same product).
    21 evac instructions per supertile, alternated ACT/DVE (11/10).
  - fp8e4 DoubleRow (measured same 216ns issue gap as fp16, 2x contraction):
      * L2 main-path accumulation: teacher pairs (rh1_k, rh1_k+1) -> 2 DR
        matmuls instead of 4 (per head per half).
      * L3 teachers: block-diagonal weights pair (rh2_h0_k, rh2_h1_k) across
        the two heads -> 4 full-width DR matmuls replace 8.
  - Teacher activations rh1/rh2 stored fp8e4 (scaled s1/s2); main path
    (xu, h1, h2, h3, all L1, L2t weights, L3 mains, L4) stays fp16.
    L2t matmuls run mixed fp16-weights x fp8-moving (measured full speed).
  - All scales are powers of two folded into host-packed weights/biases, so
    every evacuation is exactly relu(psum + bias): ACT activation(Relu) and
    DVE tensor_scalar(add,max) are interchangeable.
  - L3 mains: two 64-out fp16 matmuls col-paired via tile_position (measured
    fully concurrent). L4: 16-col block-diag variants accumulate 8 half-tiles
    into one PSUM bank; one small ACT copy (folding 1/S3) + DMA per 8.
  - Full-batch numpy-emulated rel err of this exact scheme: 5.6e-3.
"""

import os
import sys
from contextlib import ExitStack

import ml_dtypes
import numpy as np

for _p in ("/opt/trn_rl_repo",):
    if _p not in sys.path and os.path.isdir(_p):
        sys.path.insert(0, _p)

import concourse.bass as bass
import concourse.tile as tile
from concourse import bacc, mybir
from concourse._compat import with_exitstack
from concourse.bass import ts
from concourse.bass_utils import run_bass_kernel_spmd

# Problem constants (hardcoded; kernel.py must be self-contained).
B = 262144
NCORES = 8
CB = B // NCORES          # rows per core
ST = 1024                 # supertile rows (2 PSUM banks per product)
NT = 512                  # matmul moving width (one PSUM bank)
H1 = 128
H2 = 64
K = 4

F32 = mybir.dt.float32
F16 = mybir.dt.float16
F8 = mybir.dt.float8e4
E4NP = ml_dtypes.float8_e4m3
AF = mybir.ActivationFunctionType
ALU = mybir.AluOpType
PM = mybir.MatmulPerfMode

# power-of-two scales (folded host-side; see docstring)
S1 = 16.0   # rh1 storage scale
S2 = 16.0   # rh2 storage scale
SM = 128.0  # h2 storage scale (L2 main accum)
S3 = 128.0  # h3 storage scale


# ---------------------------------------------------------------------------
# fp16 weight column layout: [128, N16COLS]
# ---------------------------------------------------------------------------
def _w16layout():
    off = {}
    cur = 0
    for h in (0, 1):
        for k in range(K):
            off[f"l1t{h}{k}"] = (cur, 128)
            cur += 128
        off[f"l1m{h}"] = (cur, 128)
        cur += 128
        for k in range(K):
            off[f"l2t{h}{k}"] = (cur, 128)
            cur += 128
        off[f"l2m{h}"] = (cur, 128)
        cur += 128
        off[f"l3m{h}"] = (cur, 64)
        cur += 64
    off["l4"] = (cur, 8 * 16)
    cur += 8 * 16
    return off, cur


W16OFF, N16COLS = _w16layout()

# fp8 weight slots: [128, 8, 2, 128]; slots 0-3 = wacc(h0p0,h0p1,h1p0,h1p1),
# slots 4-7 = w3t k=0..3 (block-diag over heads)
N8SLOTS = 8


def _blayout():
    off = {}
    cur = 0
    for h in (0, 1):
        for k in range(K):
            off[f"b1t{h}{k}"] = cur
            cur += 1
        off[f"b1m{h}"] = cur
        cur += 1
        for k in range(K):
            off[f"b2t{h}{k}"] = cur
            cur += 1
        off[f"b2m{h}"] = cur
        cur += 1
    off["b3cat"] = cur
    cur += 1
    return off, cur


BOFF, BCOLS = _blayout()


# ---------------------------------------------------------------------------
# Host-side parameter folding
# ---------------------------------------------------------------------------
def _q8(x):
    return np.asarray(x, np.float32).astype(E4NP)


def prepare_params(inputs):
    """Returns (w16 [128,N16COLS] f16, w8 [128,8,2,128] f8e4,
    biasv [128,BCOLS] f32, (b4, b8))."""
    m = np.float32(np.asarray(inputs["mix_factor"]).reshape(-1)[0])
    P = np.asarray(inputs["teacher_P"], np.float32).reshape(K)
    om = np.float32(1.0) - m
    c = m * P  # [K], >= 0

    w16 = np.zeros((128, N16COLS), np.float32)
    w8 = np.zeros((128, N8SLOTS, 2, 128), np.float32)
    biasv = np.zeros((128, BCOLS), np.float32)

    def wput(name, arr):
        o, wd = W16OFF[name]
        arr = np.asarray(arr, np.float32)
        assert arr.shape[1] == wd, (name, arr.shape, wd)
        w16[: arr.shape[0], o : o + wd] = arr

    def bput(name, vec, parts=slice(0, 128)):
        vec = np.asarray(vec, np.float32).reshape(-1)
        biasv[parts, BOFF[name]] = vec

    heads = [
        ("W1", "b1", "W2", "b2", "W3", "b3", "W4", "b4", "tW1", "tb1", "tW2", "tb2", "tW3", "tb3"),
        ("W5", "b5", "W6", "b6", "W7", "b7", "W8", "b8", "tW5", "tb5", "tW6", "tb6", "tW7", "tb7"),
    ]
    out_biases = []
    l4 = np.zeros((128, 8 * 16), np.float32)
    for h, names in enumerate(heads):
        (Wa, ba, Wb, bb, Wc, bc, Wd, bd, tWa, tba, tWb, tbb, tWc, tbc) = (
            np.asarray(inputs[n], np.float32) for n in names
        )
        # L1 teachers: weights pre-scaled s1 (psum = s1*z1nb); bias s1*tba.
        for k in range(K):
            wput(f"l1t{h}{k}", (S1 * tWa[k]).T)
            bput(f"b1t{h}{k}", S1 * tba[k])
        # L1 main folded (mixsum of layer 1 is linear in xu).
        W1eff = om * Wa + m * np.einsum("k,koi->oi", P, tWa)
        b1eff = om * ba + m * (P[:, None] * tba).sum(0)
        wput(f"l1m{h}", W1eff.T)
        bput(f"b1m{h}", b1eff)
        # L2 teachers: f16 weights (s2/s1 * c_k tWb) x fp8 rh1; bias s2*c_k*tbb.
        for k in range(K):
            wput(f"l2t{h}{k}", ((S2 / S1) * c[k] * tWb[k]).T)
            bput(f"b2t{h}{k}", S2 * c[k] * tbb[k])
        # L2 main: f16 (SM om Wb) @ h1 + DR fp8 pairs q8(SM c_k tWb / s1) @ rh1.
        wput(f"l2m{h}", (SM * om * Wb).T)
        bput(f"b2m{h}", SM * (om * bb + (c[:, None] * tbb).sum(0)))
        for p in range(2):
            for s in range(2):
                k = 2 * p + s
                w8[:, 2 * h + p, s, :] = (SM * c[k] * tWb[k] / S1).T
        # L3 main: f16 (S3 om Wc / SM) @ (SM h2); stored h3 = S3*h3.
        wput(f"l3m{h}", (S3 * om * Wc / SM).T)
        b3eff = om * bc + m * (P[:, None] * tbc).sum(0)
        bput("b3cat", S3 * b3eff, parts=slice(64 * h, 64 * h + 64))
        # L3 teachers block-diag: slot 4+k slice h covers head h's 64 outputs.
        for k in range(K):
            w8[:, 4 + k, h, 64 * h : 64 * h + 64] = (S3 * tWc[k] / S2).T
        # L4 variants (j = half-tile % 8): psum = S3 * y_partial.
        for j in range(8):
            l4[64 * h : 64 * h + 64, j * 16 + 8 * h + j] = Wd[0]
        out_biases.append(np.float32(bd[0]))

    o, wd = W16OFF["l4"]
    w16[:, o : o + wd] = l4
    return w16.astype(np.float16), _q8(w8), biasv, out_biases


def prepare_xut(inputs):
    x = np.asarray(inputs["x"], np.float32)
    u = np.asarray(inputs["u"], np.float32)
    xu = np.concatenate([x, u], axis=1)  # [B, 128]
    return np.ascontiguousarray(xu.T).astype(np.float16)  # [128, B]


# ---------------------------------------------------------------------------
# Kernel body
# ---------------------------------------------------------------------------
@with_exitstack
def _critic_body(ctx: ExitStack, tc, out_ap, xu_ap, w16_ap, w8_ap, bias_ap, sts: int):
    nc = tc.nc

    const = ctx.enter_context(tc.tile_pool(name="const", bufs=1))
    xup = ctx.enter_context(tc.tile_pool(name="xup", bufs=4))
    actp = ctx.enter_context(tc.tile_pool(name="actp", bufs=3))
    psp = ctx.enter_context(tc.tile_pool(name="psp", bufs=4, space=bass.MemorySpace.PSUM))

    w16 = const.tile([128, N16COLS], F16)
    nc.gpsimd.dma_start(w16[:], w16_ap[:])
    w8 = const.tile([128, N8SLOTS, 2, 128], F8)
    nc.gpsimd.dma_start(w8[:], w8_ap[:])
    biasv = const.tile([128, BCOLS], F32)
    nc.gpsimd.dma_start(biasv[:], bias_ap[:])

    def w(name):
        o, wd = W16OFF[name]
        return w16[:, o : o + wd]

    def bvec(name):
        col = BOFF[name]
        return biasv[:, col : col + 1]

    def evac(eng, dst, src, bname):
        # dst = relu(src + bias)
        if eng == 0:
            nc.scalar.activation(dst, src, AF.Relu, bias=bvec(bname), scale=1.0)
        else:
            nc.vector.tensor_scalar(
                out=dst, in0=src, scalar1=bvec(bname), scalar2=0.0,
                op0=ALU.add, op1=ALU.max,
            )

    l4o, _ = W16OFF["l4"]
    HALF = (slice(0, NT), slice(NT, ST))

    def front_items(t, tl):
        """18 fast products (L1 + L2t) of supertile t, as emission closures."""
        xu, rh1p, rh2p, h1t = tl["xu"], tl["rh1p"], tl["rh2p"], tl["h1t"]

        def l1t(h, k):
            def emit(eng):
                ps = psp.tile([128, ST], F32, tag="ps")
                for hf in (0, 1):
                    nc.tensor.matmul(ps[:, HALF[hf]], w(f"l1t{h}{k}"), xu[:, HALF[hf]],
                                     start=True, stop=True)
                evac(eng, rh1p[:, 4 * h + k, :], ps[:], f"b1t{h}{k}")
            return emit

        def l1m(h):
            def emit(eng):
                ps = psp.tile([128, ST], F32, tag="ps")
                for hf in (0, 1):
                    nc.tensor.matmul(ps[:, HALF[hf]], w(f"l1m{h}"), xu[:, HALF[hf]],
                                     start=True, stop=True)
                evac(eng, h1t[:, h, :], ps[:], f"b1m{h}")
            return emit

        def l2t(h, k):
            def emit(eng):
                ps = psp.tile([128, ST], F32, tag="ps")
                for hf in (0, 1):
                    nc.tensor.matmul(ps[:, HALF[hf]], w(f"l2t{h}{k}"),
                                     rh1p[:, 4 * h + k, HALF[hf]], start=True, stop=True)
                evac(eng, rh2p[:, 4 * h + k, :], ps[:], f"b2t{h}{k}")
            return emit

        items = []
        for h in (0, 1):
            items += [l1t(h, k) for k in range(K)] + [l1m(h)]
        for h in (0, 1):
            items += [l2t(h, k) for k in range(K)]
        return items

    def back_items(t, tl):
        """4 slow products (L2 accum x2, L3, L4) of supertile t."""
        rh1p, rh2p, h1t, h2t, h3t = (tl["rh1p"], tl["rh2p"], tl["h1t"], tl["h2t"], tl["h3t"])

        def l2acc(h):
            def emit(eng):
                ps = psp.tile([128, ST], F32, tag="ps")
                for hf in (0, 1):
                    nc.tensor.matmul(ps[:, HALF[hf]], w(f"l2m{h}"), h1t[:, h, HALF[hf]],
                                     start=True, stop=False)
                    for p in range(2):
                        nc.tensor.matmul(ps[:, HALF[hf]], w8[:, 2 * h + p],
                                         rh1p[:, 4 * h + 2 * p : 4 * h + 2 * p + 2, HALF[hf]],
                                         start=False, stop=(p == 1), perf_mode=PM.DoubleRow)
                evac(eng, h2t[:, h, :], ps[:], f"b2m{h}")
            return emit

        def l3(eng):
            ps3 = psp.tile([128, ST], F32, tag="ps")
            for hf in (0, 1):
                # full-width DR teachers carry the (tracked) start/stop flags;
                # col-paired mains are skip_group_check and accumulate after.
                for k in range(K):
                    nc.tensor.matmul(ps3[:, HALF[hf]], w8[:, 4 + k],
                                     rh2p[:, k :: 4, HALF[hf]],
                                     start=(k == 0), stop=(k == K - 1), perf_mode=PM.DoubleRow)
                for h in (0, 1):
                    nc.tensor.matmul(ps3[64 * h : 64 * h + 64, HALF[hf]], w(f"l3m{h}"),
                                     h2t[:, h, HALF[hf]], start=False, stop=False,
                                     tile_position=(0, 64 * h), skip_group_check=True)
            evac(eng, h3t[:], ps3[:], "b3cat")

        def l4(eng):
            ps4 = psp.tile([128, ST], F32, tag="ps")
            for hf in (0, 1):
                nc.tensor.matmul(ps4[0:16, HALF[hf]], w16[:, l4o : l4o + 16],
                                 h3t[:, HALF[hf]], start=True, stop=True)
            o = actp.tile([16, ST], F32, tag="osb")
            if eng == 0:
                nc.scalar.activation(o[:], ps4[0:16, :], AF.Copy, bias=0.0, scale=1.0 / S3)
            else:
                nc.vector.tensor_scalar(out=o[:], in0=ps4[0:16, :], scalar1=1.0 / S3,
                                        scalar2=None, op0=ALU.mult)
            nc.gpsimd.dma_start(out_ap[:, ts(t, ST)], o[:])

        return [l2acc(0), l2acc(1), l3, l4]

    # Software pipeline: interleave supertile t-1's slow back products into
    # supertile t's fast front stream so product creation and evacuation rates
    # stay matched and neither PE nor the evac engines run dry.
    # Engine pattern per 22-product cycle: ACT=0 x12, DVE=1 x10 (ACT op is
    # ~13% cheaper, so it takes two more products).
    ENG = [0, 1, 0, 1, 0, 1, 0, 1, 0, 1, 0, 1, 0, 1, 0, 1, 0, 1, 0, 1, 0, 0]
    BACK_POS = {3: 0, 8: 1, 12: 2, 16: 3}  # after front item i -> back item j

    prev = None
    for t in range(sts + 1):
        if t < sts:
            tl = {
                "xu": xup.tile([128, ST], F16, tag="xu", name="xu"),
                "rh1p": actp.tile([128, 8, ST], F8, tag="rh1p", name="rh1p"),
                "rh2p": actp.tile([128, 8, ST], F8, tag="rh2p", name="rh2p"),
                "h1t": actp.tile([128, 2, ST], F16, tag="h1t", name="h1t"),
                "h2t": actp.tile([128, 2, ST], F16, tag="h2t", name="h2t"),
                "h3t": actp.tile([128, ST], F16, tag="h3t", name="h3t"),
            }
            nc.gpsimd.dma_start(tl["xu"][:], xu_ap[:, ts(t, ST)])
            front = front_items(t, tl)
        else:
            front = []
        back = back_items(t - 1, prev) if prev is not None else []

        ei = 0
        for i, item in enumerate(front):
            item(ENG[ei])
            ei += 1
            if i in BACK_POS and back:
                back[BACK_POS[i]](ENG[ei])
                ei += 1
        if not front:  # drain the last supertile's back half
            for item in back:
                item(ENG[ei])
                ei += 1
        prev = tl if t < sts else None


def build_nc(cb=CB):
    """Build + compile the per-core program for cb rows (cb % (8*NT) == 0)."""
    assert cb % (8 * NT) == 0
    sts = cb // ST
    nc = bacc.Bacc(
        "TRN2",
        target_bir_lowering=False,
        debug=False,
        enable_asserts=False,
        num_devices=NCORES,
    )
    xu_ap = nc.dram_tensor("xut", [128, cb], F16, kind="ExternalInput").ap()
    w16_ap = nc.dram_tensor("w16", [128, N16COLS], F16, kind="ExternalInput").ap()
    w8_ap = nc.dram_tensor("w8", [128, N8SLOTS, 2, 128], F8, kind="ExternalInput").ap()
    bias_ap = nc.dram_tensor("biasv", [128, BCOLS], F32, kind="ExternalInput").ap()
    out_ap = nc.dram_tensor("out", [16, cb], F32, kind="ExternalOutput").ap()
    with tile.TileContext(nc) as tc:
        _critic_body(tc, out_ap, xu_ap, w16_ap, w8_ap, bias_ap, sts)
    nc.compile()
    return nc


def unscramble_out(out_c):
    """[16, cb] device layout -> (y1 [cb], y2 [cb]); row 8h = head h (S3-scaled)."""
    return [np.asarray(out_c[8 * h], np.float32) for h in (0, 1)]


_NC_CACHE = {}
LAST_RESULTS = None  # BassKernelResults of the most recent run (for profiling)


def kernel(**inputs):
    global LAST_RESULTS
    w16, w8, biasv, (b4, b8) = prepare_params(inputs)
    xut = prepare_xut(inputs)

    if CB not in _NC_CACHE:
        _NC_CACHE[CB] = build_nc(CB)
    nc = _NC_CACHE[CB]

    in_maps = [
        {
            "xut": np.ascontiguousarray(xut[:, c * CB : (c + 1) * CB]),
            "w16": w16,
            "w8": w8,
            "biasv": biasv,
        }
        for c in range(NCORES)
    ]
    res = run_bass_kernel_spmd(
        nc,
        in_maps,
        list(range(NCORES)),
        trace=bool(os.environ.get("BASS_TRACE")),
    )
    LAST_RESULTS = res

    y1 = np.empty(B, np.float32)
    y2 = np.empty(B, np.float32)
    for c in range(NCORES):
        a, b = unscramble_out(res.results[c]["out"])
        y1[c * CB : (c + 1) * CB] = a
        y2[c * CB : (c + 1) * CB] = b
    y1 += b4
    y2 += b8
    return (y1[:, None], y2[:, None])
